# revision 28
# baseline (speedup 1.0000x reference)
"""Trainium2 Bass kernel for nn_CLIP_Embedding_35613868818658.

CNN stem (3x conv1d+GroupNorm+ReLU, 768->128->256->512) -> LayerNorm ->
bidirectional Mamba (selective scan, d_inner=1024, d_state=16, L=1024) ->
out_proj + residual.  Output (2, 512, 1024) f32.

Sharding: 2 batch-groups x 4-way d_inner split (DSH=256 rows per core).
Cores 0-3 handle b=0, cores 4-7 handle b=1; core g within a group owns
d_inner rows [256g, 256(g+1)).  Cross-core traffic per group: one bf16
AllReduce of the x_dbl partials ([128,1024] = 256KB) and one bf16
AllGather of the gated scan outputs ([256,1024] per core); every core
then runs the full out_proj matmul locally (PE is idle) and writes the
final f32 output directly.

The selective scan runs as 16 (one per state index s) hardware
tensor_tensor_scan instructions per d-tile over a [128, 2048] layout that
concatenates the forward and (time-reversed) backward directions along the
free axis; a[, t=0|1024] = 0 resets the recurrence at segment starts.
The per-state y accumulation (y += C_s*h_s) and the D skip-term run on the
tensor engine as identity/diagonal matmul accumulation into PSUM; the
depthwise conv runs as 4 diagonal-matmul taps + Silu-from-PSUM.
"""

import numpy as np
import ml_dtypes

import concourse.bass as bass
import concourse.mybir as mybir
import concourse.tile as tile
from contextlib import ExitStack

BF16 = ml_dtypes.bfloat16
F32 = mybir.dt.float32
BF = mybir.dt.bfloat16

B, CIN, L = 2, 768, 1024
DM, DI, DS, DTR, DC = 512, 1024, 16, 32, 4
NCORES, NGRP = 8, 4
DSH = DI // NGRP          # 256 d_inner rows per core
NDT = DSH // 128          # 2 d-tiles of 128 partitions
T2 = 2 * L                # fwd|rev concatenated time axis
EPS = 1e-5

AluOp = mybir.AluOpType
ActFn = mybir.ActivationFunctionType


def _ap_bcast_dram(handle, offset, dims):
    """Raw AP on a DRAM tensor: dims is a list of [step, count]."""
    return bass.AP(tensor=handle, offset=offset, ap=[list(d) for d in dims])


def split_excess_waits(nc, max_waits=1):
    """Walrus rejects instructions carrying more sync waits than the ISA
    encoding has slots for (1 on this toolchain).  Move excess waits onto
    preceding same-engine NoOps."""
    for bb in nc.main_func.blocks:
        insts = bb.instructions
        out, changed = [], False
        for ins in insts:
            si = ins.sync_info
            if si is not None and si.on_wait is not None and len(si.on_wait) > max_waits:
                waits = list(si.on_wait)
                keep, rest = waits[:max_waits], waits[max_waits:]
                idx = 0
                while rest:
                    chunk, rest = rest[:max_waits], rest[max_waits:]
                    nop = mybir.InstNoOp(
                        name=f"{ins.name}-wsplit{idx}",
                        engine=ins.engine,
                        sync_info=mybir.SyncInfo(on_wait=chunk, on_update=[]),
                        bass_nofuse=True,
                    )
                    out.append(nop)
                    idx += 1
                ins.sync_info = mybir.SyncInfo(
                    on_wait=keep, on_update=list(si.on_update or [])
                )
                changed = True
            out.append(ins)
        if changed:
            bb.instructions = out


def build_program(a_vals, split_waits=True, reps=1):
    """Build the SPMD Bass program.  a_vals: 16 negative floats, A[s] = -(s+1)
    (verified d-independent and equal for both directions on the host)."""
    nc = bass.Bass("TRN2", target_bir_lowering=False, debug=False,
                   num_devices=NCORES)

    dt_in = lambda n, s, d=BF: nc.dram_tensor(n, list(s), d, kind="ExternalInput")

    x_in = dt_in("x", (CIN, L + 2))                      # host-padded, bf16
    w1T = dt_in("w1T", (3, 6, 128, 128))
    w2T = dt_in("w2T", (3, 1, 128, 256))
    w3T = dt_in("w3T", (3, 2, 128, 512))
    cb1 = dt_in("cb1", (128, 1), F32)
    cb2 = dt_in("cb2", (256, 1), F32)
    cb3 = dt_in("cb3", (512, 1), F32)
    gng1 = dt_in("gng1", (128, 1), F32)
    gnb1 = dt_in("gnb1", (128, 1), F32)
    gng2 = dt_in("gng2", (256, 1), F32)
    gnb2 = dt_in("gnb2", (256, 1), F32)
    gng3 = dt_in("gng3", (512, 1), F32)
    gnb3 = dt_in("gnb3", (512, 1), F32)
    onehot = dt_in("onehot", (3, 128, 32))
    onehotT = dt_in("onehotT", (3, 32, 128), F32)
    ones_col = dt_in("ones_col", (128, 1))
    inprojT = dt_in("inprojT", (4, 128, 512))
    augT = dt_in("augT", (2, 512))
    xpT = dt_in("xpT", (2, 2, 128, 64))                 # [dir][ktile]
    dtT = dt_in("dtT", (2, 32, 256))                    # [dir]
    ndtb = dt_in("ndtb", (2, 256, 1), F32)              # -dt_b
    cvdg = dt_in("cvdg", (2, 2, 4, 128, 128))           # [dir][dt][tap] diag
    cvbdg = dt_in("cvbdg", (2, 2, 128, 128))            # [dir][dt] diag(cv_b)
    Ddg = dt_in("Ddg", (2, 2, 128, 128))                # [dir][dt] diag(D)
    ident = dt_in("ident", (128, 128))
    outT = dt_in("outT", (8, 128, 512))                 # full d_inner

    out_ext = nc.dram_tensor("out", [DM, L], F32, kind="ExternalOutput")

    with tile.TileContext(nc) as tc, ExitStack() as ctx:
        P = 128
        consts = ctx.enter_context(tc.tile_pool(name="consts", bufs=1))
        mid = ctx.enter_context(tc.tile_pool(name="mid", bufs=1))
        dram = ctx.enter_context(tc.tile_pool(name="dram", bufs=1, space="DRAM"))
        sync, vec, pool, act, pe = nc.sync, nc.vector, nc.gpsimd, nc.scalar, nc.tensor

        # ---------------- consts to SBUF ----------------
        def load(poolh, shape, src, dtype=BF, name=None):
            t = poolh.tile(list(shape), dtype, tag=name)
            sync.dma_start(t[:], src)
            return t

        w1 = [[load(consts, (P, 128), w1T[k, ct], name=f"w1_{k}_{ct}")
               for ct in range(6)] for k in range(3)]
        w2 = [[load(consts, (P, 256), w2T[k, ct], name=f"w2_{k}_{ct}")
               for ct in range(1)] for k in range(3)]
        w3 = [[load(consts, (P, 512), w3T[k, ct], name=f"w3_{k}_{ct}")
               for ct in range(2)] for k in range(3)]
        def load_cols(dramt, co, name, width=1):
            return [load(consts, (128, width), dramt[mt * 128:(mt + 1) * 128, :],
                         F32, f"{name}{mt}") for mt in range(co // 128)]

        cbs = [load_cols(cb1, 128, "cb1"), load_cols(cb2, 256, "cb2"),
               load_cols(cb3, 512, "cb3")]
        gngs = [load_cols(gng1, 128, "gng1"), load_cols(gng2, 256, "gng2"),
                load_cols(gng3, 512, "gng3")]
        gnbs = [load_cols(gnb1, 128, "gnb1"), load_cols(gnb2, 256, "gnb2"),
                load_cols(gnb3, 512, "gnb3")]
        oneh = [load(consts, (P, 32), onehot[i], name=f"onehot{i}")
                for i in range(3)]
        ohT = [load(consts, (32, 128), onehotT[i], F32, name=f"onehotT{i}")
               for i in range(3)]
        ones1 = load(consts, (P, 1), ones_col[:], name="ones1")
        ones_1xP = consts.tile([1, P], BF, tag="ones_1xP")
        vec.memset(ones_1xP[:], 1.0)
        ipT = [load(consts, (P, 512), inprojT[kt], name=f"ipT{kt}") for kt in range(4)]
        augTs = load(consts, (2, 512), augT[:], name="augT")
        xpTs = [[load(consts, (P, 64), xpT[d, kt], name=f"xpT{d}{kt}")
                 for kt in range(2)] for d in range(2)]
        dtTs = [load(consts, (32, 256), dtT[d], name=f"dtT{d}") for d in range(2)]
        ndtbs = [[load(consts, (128, 1), ndtb[d, dt * 128:(dt + 1) * 128, :], F32,
                       f"ndtb{d}{dt}") for dt in range(2)] for d in range(2)]
        cvds = [[[load(consts, (P, 128), cvdg[d, dt, k], name=f"cvd{d}{dt}{k}")
                  for k in range(4)] for dt in range(2)] for d in range(2)]
        cvbds = [[load(consts, (P, 128), cvbdg[d, dt], name=f"cvbd{d}{dt}")
                  for dt in range(2)] for d in range(2)]
        ones_row = consts.tile([P, 512], BF, tag="ones_row")
        vec.memset(ones_row[:], 1.0)
        Ddgs = [[load(consts, (P, 128), Ddg[d, dt], name=f"Ddg{d}{dt}")
                 for dt in range(2)] for d in range(2)]
        idn = load(consts, (P, 128), ident[:], name="ident")
        outTs = [load(consts, (P, 512), outT[j], name=f"outT{j}") for j in range(8)]

        epsc = consts.tile([128, 1], F32, tag="epsc")
        vec.memset(epsc[:], EPS)

        # DRAM scratch
        xdbl_loc = dram.tile([128, L], BF, tag="xdbl_loc")
        xdbl_red = dram.tile([128, L], BF, tag="xdbl_red")
        ygl = dram.tile([2 * P, L], BF, tag="ygl")
        ygall = dram.tile([NGRP * 2 * P, L], BF, tag="ygall")

        for rep in range(reps):
            fctx = ExitStack()
            psum = fctx.enter_context(tc.tile_pool(name=f"psum{rep}", bufs=2,
                                                   space="PSUM"))
            stem = fctx.enter_context(tc.tile_pool(name=f"stem{rep}", bufs=1))
            stemtmp = fctx.enter_context(tc.tile_pool(name=f"stemtmp{rep}", bufs=3))
            statp = fctx.enter_context(tc.tile_pool(name=f"statp{rep}", bufs=2))
            rows = fctx.enter_context(tc.tile_pool(name=f"rows{rep}", bufs=1))
            x_t = [load(stem, (P, L + 2), x_in[ct * P:(ct + 1) * P, :],
                        name=f"x{ct}") for ct in range(6)]
            # ---------------- CNN stem ----------------
            def conv_gn_relu(layer, in_tiles, ws, cb, gng, gnb, co, out_f32):
                """in_tiles: list of padded (128, L+2) bf16; returns list of
                normalized+relu'd output tiles.  out_f32: emit f32 (for res)."""
                n_ct = len(in_tiles)
                n_co = co // 128
                cg = co // 32            # channels per group
                ngt = 128 // cg          # groups per 128-channel tile
                group_elems = float(cg) * L
                outs = []
                for mt in range(n_co):
                    h_raw = stemtmp.tile([P, L], F32, tag="h_raw")
                    stat4 = statp.tile([P, 4], F32, tag="stat4")
                    sq = stemtmp.tile([P, 512], BF, tag="sq")
                    for n in range(2):
                        ps = psum.tile([P, 512], F32, tag="ps_main", name="ps")
                        nmm = n_ct * 3
                        i = 0
                        for ct in range(n_ct):
                            for k in range(3):
                                pe.matmul(
                                    ps[:],
                                    ws[k][ct][:, mt * 128:(mt + 1) * 128],
                                    in_tiles[ct][:, n * 512 + k: n * 512 + k + 512],
                                    start=(i == 0), stop=(i == nmm - 1),
                                )
                                i += 1
                        act.activation(h_raw[:, n * 512:(n + 1) * 512], ps[:],
                                       ActFn.Identity, bias=cb[mt][:],
                                       accum_out=stat4[:, n:n + 1])
                        act.activation(sq[:], h_raw[:, n * 512:(n + 1) * 512],
                                       ActFn.Square, accum_out=stat4[:, 2 + n:3 + n])
                    # group stats: per-partition sums -> per-group via one-hot matmul
                    stat4b = statp.tile([P, 4], BF, tag="stat4b")
                    vec.tensor_copy(stat4b[:], stat4[:])
                    gps = psum.tile([32, 4], F32, tag="ps_small", name="gps", bufs=2)
                    pe.matmul(gps[:], oneh[layer - 1][:], stat4b[:])
                    gsb = statp.tile([32, 4], F32, tag="gsb")
                    vec.tensor_copy(gsb[:], gps[:])
                    sx = statp.tile([32, 1], F32, tag="sx")
                    sq_g = statp.tile([32, 1], F32, tag="sq_g")
                    vec.tensor_add(sx[:], gsb[:, 0:1], gsb[:, 1:2])
                    vec.tensor_add(sq_g[:], gsb[:, 2:3], gsb[:, 3:4])
                    mean = statp.tile([32, 1], F32, tag="mean")
                    vec.tensor_scalar_mul(mean[:], sx[:], 1.0 / group_elems)
                    msq = statp.tile([32, 1], F32, tag="msq")
                    vec.tensor_mul(msq[:], mean[:], mean[:])
                    var = statp.tile([32, 1], F32, tag="var")
                    vec.scalar_tensor_tensor(var[:], sq_g[:], 1.0 / group_elems, msq[:],
                                             AluOp.mult, AluOp.subtract)
                    sig_g = statp.tile([32, 1], F32, tag="sig_g")
                    act.activation(sig_g[:], var[:], ActFn.Sqrt, bias=epsc[:32, :])
                    rstd = statp.tile([32, 1], F32, tag="rstd")
                    vec.reciprocal(rstd[:], sig_g[:])
                    # pack [rstd, mean] and expand groups 32 -> channels 128
                    # via a one-hot-transpose matmul (no DRAM round trip)
                    stat2 = statp.tile([32, 2], F32, tag="stat2")
                    vec.tensor_copy(stat2[:, 0:1], rstd[:])
                    vec.tensor_copy(stat2[:, 1:2], mean[:])
                    gch = psum.tile([P, 2], F32, tag="ps_bc", name="gch", bufs=2)
                    pe.matmul(gch[:], ohT[layer - 1][:], stat2[:])
                    ch2 = statp.tile([P, 2], F32, tag="ch2")
                    act.activation(ch2[:], gch[:], ActFn.Copy)
                    scale_c = statp.tile([P, 1], F32, tag="scale_c")
                    vec.tensor_mul(scale_c[:], ch2[:, 0:1], gng[mt][:])
                    nmean_s = statp.tile([P, 1], F32, tag="nmean_s")
                    vec.tensor_mul(nmean_s[:], ch2[:, 1:2], scale_c[:])
                    bias_c = statp.tile([P, 1], F32, tag="bias_c")
                    vec.tensor_sub(bias_c[:], gnb[mt][:], nmean_s[:])
                    if out_f32:
                        h_out = mid.tile([P, L], F32, tag=f"res{mt}")
                        act.activation(h_out[:], h_raw[:], ActFn.Relu,
                                       scale=scale_c[:], bias=bias_c[:])
                    else:
                        h_out = stem.tile([P, L + 2], BF, tag=f"h{layer}_{mt}")
                        vec.memset(h_out[:, 0:1], 0.0)
                        vec.memset(h_out[:, L + 1:L + 2], 0.0)
                        act.activation(h_out[:, 1:L + 1], h_raw[:], ActFn.Relu,
                                       scale=scale_c[:], bias=bias_c[:])
                    outs.append(h_out)
                return outs

            h1 = conv_gn_relu(1, x_t, w1, cbs[0], gngs[0], gnbs[0], 128, False)
            h2 = conv_gn_relu(2, h1, w2, cbs[1], gngs[1], gnbs[1], 256, False)
            res = conv_gn_relu(3, h2, w3, cbs[2], gngs[2], gnbs[2], 512, True)

            h3b = []
            for mt in range(4):
                t = stem.tile([P, L], BF, tag=f"h3b{mt}")
                vec.tensor_copy(t[:], res[mt][:])
                h3b.append(t)

            # ---------------- LayerNorm stats (over channels, via matmuls) -------
            hsq = []
            for mt in range(4):
                t = stemtmp.tile([P, L], BF, tag="hsq")
                vec.tensor_mul(t[:], h3b[mt][:], h3b[mt][:])
                hsq.append(t)
            musum = rows.tile([1, L], F32, tag="musum")
            sqsum = rows.tile([1, L], F32, tag="sqsum")
            for n in range(2):
                mu_ps = psum.tile([1, 512], F32, tag="ps_row", name="mu_ps", bufs=2)
                for kt in range(4):
                    pe.matmul(mu_ps[:], ones1[:],
                              h3b[kt][:, n * 512:(n + 1) * 512],
                              start=(kt == 0), stop=(kt == 3))
                act.activation(musum[:, n * 512:(n + 1) * 512], mu_ps[:], ActFn.Copy)
                sq_ps = psum.tile([1, 512], F32, tag="ps_row", name="sq_ps", bufs=2)
                for kt in range(4):
                    pe.matmul(sq_ps[:], ones1[:],
                              hsq[kt][:, n * 512:(n + 1) * 512],
                              start=(kt == 0), stop=(kt == 3))
                act.activation(sqsum[:, n * 512:(n + 1) * 512], sq_ps[:], ActFn.Copy)
            nmu = rows.tile([1, L], F32, tag="nmu")
            vec.tensor_scalar_mul(nmu[:], musum[:], -1.0 / DM)
            msql = rows.tile([1, L], F32, tag="msql")
            act.activation(msql[:], musum[:], ActFn.Square, scale=1.0 / DM)
            varl = rows.tile([1, L], F32, tag="varl")
            vec.scalar_tensor_tensor(varl[:], sqsum[:], 1.0 / DM, msql[:],
                                     AluOp.mult, AluOp.subtract)
            sigma = rows.tile([1, L], F32, tag="sigma")
            act.activation(sigma[:], varl[:], ActFn.Sqrt, bias=epsc[:1, :])
            recip = rows.tile([1, L], F32, tag="recip")
            vec.reciprocal(recip[:], sigma[:])
            nmu_b = rows.tile([1, L], BF, tag="nmu_b")
            vec.tensor_copy(nmu_b[:], nmu[:])
            sig_b = rows.tile([1, L], BF, tag="sig_b")
            vec.tensor_copy(sig_b[:], sigma[:])
            aug = rows.tile([2, L], BF, tag="aug")
            sync.dma_start(aug[0:1, :], nmu_b[:])
            sync.dma_start(aug[1:2, :], sig_b[:])
            recip_b = rows.tile([1, L], BF, tag="recip_b")
            vec.tensor_copy(recip_b[:], recip[:])
            rbc = rows.tile([P, L], BF, tag="rbc")
            for n in range(2):
                rps = psum.tile([P, 512], F32, tag="ps_main", name="rps")
                pe.matmul(rps[:], ones_1xP[:], recip_b[:, n * 512:(n + 1) * 512])
                act.activation(rbc[:, n * 512:(n + 1) * 512], rps[:], ActFn.Copy)

            # ---------------- in_proj (LN folded in) ----------------
            # xpad[dt]: (128, L+6) bf16, 3 zero cols each side; z[dt]: (128, L)
            xpad = []
            zt = []
            for dt in range(NDT):
                xp_ = mid.tile([P, L + 6], BF, tag=f"xpad{dt}")
                vec.memset(xp_[:, 0:3], 0.0)
                vec.memset(xp_[:, L + 3:L + 6], 0.0)
                xpad.append(xp_)
                zt.append(mid.tile([P, L], BF, tag=f"z{dt}", name=f"z{dt}"))
            for m in range(4):
                for n in range(2):
                    ps = psum.tile([P, 512], F32, tag="ps_main", name="ps")
                    for kt in range(4):
                        pe.matmul(ps[:], ipT[kt][:, m * 128:(m + 1) * 128],
                                  h3b[kt][:, n * 512:(n + 1) * 512],
                                  start=(kt == 0), stop=False)
                    pe.matmul(ps[:], augTs[:, m * 128:(m + 1) * 128],
                              aug[:, n * 512:(n + 1) * 512], start=False, stop=True)
                    if m < 2:
                        dst = xpad[m][:, 3 + n * 512: 3 + (n + 1) * 512]
                    else:
                        dst = zt[m - 2][:, n * 512:(n + 1) * 512]
                    vec.tensor_mul(dst, ps[:], rbc[:, n * 512:(n + 1) * 512])

            fctx.close()  # free stem/LN scratch (incl. psum) for the scan phase
            s1ctx = ExitStack()
            psum1 = s1ctx.enter_context(tc.tile_pool(name=f"psum1_{rep}", bufs=1,
                                                     space="PSUM"))
            scanp = s1ctx.enter_context(tc.tile_pool(name=f"scanp{rep}", bufs=2))
            onep = s1ctx.enter_context(tc.tile_pool(name=f"onep{rep}", bufs=1))

            # ------- depthwise causal conv (PE diag taps) + silu-from-PSUM -------
            u_cat = [mid.tile([P, T2], BF, tag=f"u{dt}", name=f"u{dt}")
                     for dt in range(NDT)]
            for dt in range(NDT):
                for d in range(2):  # 0 = fwd, 1 = rev (tau domain)
                    pdw = psum1.tile([P, L], F32, tag="ps_dw", name="pdw", bufs=2)
                    sg = scanp.tile([P, L], BF, tag="dwsg")
                    for c in range(2):
                        pe.matmul(pdw[:, c * 512:(c + 1) * 512],
                                  cvbds[d][dt][:], ones_row[:],
                                  start=True, stop=False)
                        for k in range(4):
                            off = (k if d == 0 else 3 - k) + c * 512
                            pe.matmul(pdw[:, c * 512:(c + 1) * 512],
                                      cvds[d][dt][k][:],
                                      xpad[dt][:, off:off + 512],
                                      start=False, stop=(k == 3))
                        act.activation(sg[:, c * 512:(c + 1) * 512],
                                       pdw[:, c * 512:(c + 1) * 512],
                                       ActFn.Sigmoid)
                    if d == 0:
                        vec.tensor_mul(u_cat[dt][:, 0:L], pdw[:], sg[:])
                    else:
                        tmpv = scanp.tile([P, L], BF, tag="dwtmp")
                        vec.tensor_mul(tmpv[:], pdw[:], sg[:])
                        vec.tensor_copy(u_cat[dt][:, L:T2], tmpv[:, L - 1::-1])

            # ---------------- x_dbl projection + bf16 AllReduce ----------------
            xsb = onep.tile([128, L], BF, tag="xsb")
            for d in range(2):
                for n in range(2):
                    xps = psum1.tile([64, 512], F32, tag="ps_xp", name="xps")
                    for dt in range(NDT):
                        pe.matmul(xps[:], xpTs[d][dt][:],
                                  u_cat[dt][:, d * L + n * 512: d * L + (n + 1) * 512],
                                  start=(dt == 0), stop=(dt == 1))
                    act.activation(xsb[64 * d:64 * d + 64, n * 512:(n + 1) * 512],
                                   xps[:], ActFn.Copy)
            sync.dma_start(xdbl_loc[:], xsb[:])
            pool.collective_compute(
                "AllReduce", AluOp.add,
                replica_groups=[[0, 1, 2, 3], [4, 5, 6, 7]],
                ins=[xdbl_loc[:].opt()],
                outs=[xdbl_red[:].opt()],
            )

            # ------- dt_proj -> m = -softplus(dt @ dtw + dt_b) = ln(sigmoid(-x))
            m_cat = [mid.tile([P, T2], BF, tag=f"m{dt}", name=f"m{dt}")
                     for dt in range(NDT)]
            dtf = []
            for d in range(2):
                t = onep.tile([32, L], BF, tag=f"dtf{d}", name=f"dtf{d}")
                sync.dma_start(t[:], xdbl_red[64 * d:64 * d + 32, :])
                dtf.append(t)
            for dt in range(NDT):
                for d in range(2):
                    for n in range(2):
                        ps = psum1.tile([P, 512], F32, tag="ps_dt", name="psdt")
                        pe.matmul(ps[:], dtTs[d][:, dt * 128:(dt + 1) * 128],
                                  dtf[d][:, n * 512:(n + 1) * 512])
                        sgm = scanp.tile([P, 512], F32, tag="sgm")
                        act.activation(sgm[:], ps[:], ActFn.Sigmoid, scale=-1.0,
                                       bias=ndtbs[d][dt][:])
                        act.activation(m_cat[dt][:, d * L + n * 512: d * L + (n + 1) * 512],
                                       sgm[:], ActFn.Ln)

            # mx = -(m * u) = delta * u
            mx = [mid.tile([P, T2], BF, tag=f"mx{dt}", name=f"mx{dt}")
                  for dt in range(NDT)]
            for dt in range(NDT):
                vec.scalar_tensor_tensor(mx[dt][:], m_cat[dt][:], -1.0, u_cat[dt][:],
                                         AluOp.mult, AluOp.mult)

            # z gating (independent of the scan)
            zs = []
            for dt in range(NDT):
                sgz = scanp.tile([P, L], BF, tag="sgz")
                act.activation(sgz[:], zt[dt][:], ActFn.Sigmoid)
                t = mid.tile([P, L], BF, tag=f"zs{dt}")
                vec.tensor_mul(t[:], zt[dt][:], sgz[:])
                zs.append(t)

            s1ctx.close()
            s2ctx = ExitStack()
            scan2 = s2ctx.enter_context(tc.tile_pool(name=f"scan2_{rep}", bufs=2))
            psy_ctx = ExitStack()
            psum2 = psy_ctx.enter_context(tc.tile_pool(name=f"psum2_{rep}", bufs=1,
                                                       space="PSUM"))

            # ---------------- selective scan ----------------
            xr_ap = xdbl_red[:]
            ps_y = [psum2.tile([P, T2], F32, tag=f"ps_y{dt}", name=f"ps_y{dt}",
                               bufs=1) for dt in range(NDT)]
            for s in range(16):
                Bs = scan2.tile([P, T2], BF, tag="Bs")
                sync.dma_start(
                    Bs[:],
                    _ap_bcast_dram(xr_ap.tensor, xr_ap.offset + (32 + s) * L,
                                   [[0, P], [64 * L, 2], [1, L]]),
                )
                Cs = scan2.tile([P, T2], BF, tag="Cs")
                sync.dma_start(
                    Cs[:],
                    _ap_bcast_dram(xr_ap.tensor, xr_ap.offset + (48 + s) * L,
                                   [[0, P], [64 * L, 2], [1, L]]),
                )
                for dt in range(NDT):
                    a_s = scan2.tile([P, T2], BF, tag="a_s")
                    act.activation(a_s[:], m_cat[dt][:], ActFn.Exp,
                                   scale=float(-a_vals[s]))
                    vec.memset(a_s[:, 0:1], 0.0)
                    vec.memset(a_s[:, L:L + 1], 0.0)
                    b_s = scan2.tile([P, T2], BF, tag="b_s")
                    bs_eng = pool if dt == 0 else vec
                    bs_eng.tensor_mul(b_s[:], mx[dt][:], Bs[:])
                    h_s = scan2.tile([P, T2], BF, tag="h_s")
                    vec.tensor_tensor_scan(h_s[:], a_s[:], b_s[:], 0.0,
                                           AluOp.mult, AluOp.add)
                    gs = scan2.tile([P, T2], BF, tag="gs")
                    pool.tensor_mul(gs[:], h_s[:], Cs[:])
                    for c in range(4):
                        pe.matmul(ps_y[dt][:, c * 512:(c + 1) * 512], idn[:],
                                  gs[:, c * 512:(c + 1) * 512],
                                  start=(s == 0), stop=False)
            # D skip-term: ps_y += diag(D_dir) @ u (fwd half / rev half)
            for dt in range(NDT):
                for c in range(4):
                    d = c // 2
                    pe.matmul(ps_y[dt][:, c * 512:(c + 1) * 512], Ddgs[d][dt][:],
                              u_cat[dt][:, c * 512:(c + 1) * 512],
                              start=False, stop=True)

            # ---------------- combine directions, gate, AllGather ----------------
            for dt in range(NDT):
                yf = scan2.tile([P, L], BF, tag="yf")
                vec.tensor_copy(yf[:], ps_y[dt][:, 0:L])
                ysum = scan2.tile([P, L], BF, tag="ysum")
                vec.tensor_add(ysum[:], yf[:], ps_y[dt][:, T2 - 1:L - 1:-1])
                yg = scan2.tile([P, L], BF, tag="yg")
                vec.tensor_mul(yg[:], ysum[:], zs[dt][:])
                sync.dma_start(ygl[dt * P:(dt + 1) * P, :], yg[:])
            psy_ctx.close()
            psum3 = s2ctx.enter_context(tc.tile_pool(name=f"psum3_{rep}", bufs=1,
                                                     space="PSUM"))
            pool.collective_compute(
                "AllGather", AluOp.bypass,
                replica_groups=[[0, 1, 2, 3], [4, 5, 6, 7]],
                ins=[ygl[:].opt()],
                outs=[ygall[:].opt()],
            )

            # ---------------- full out_proj + residual on every core -----------
            # j-outer accumulation into 8 open PSUM groups so matmuls start as
            # soon as each gathered d-slice lands.
            ygf = [scan2.tile([P, L], BF, tag=f"ygf{j}", name=f"ygf{j}", bufs=1)
                   for j in range(8)]
            for j in range(8):
                eng = sync if j % 2 == 0 else act
                eng.dma_start(ygf[j][:], ygall[j * P:(j + 1) * P, :])
            pss = [psum3.tile([P, 512], F32, tag=f"po{q}", name=f"po{q}", bufs=1)
                   for q in range(8)]
            for j in range(8):
                for m in range(4):
                    for n in range(2):
                        pe.matmul(pss[m * 2 + n][:],
                                  outTs[j][:, m * 128:(m + 1) * 128],
                                  ygf[j][:, n * 512:(n + 1) * 512],
                                  start=(j == 0), stop=(j == 7))
            for m in range(4):
                osb = scan2.tile([P, L], F32, tag="osb")
                for n in range(2):
                    vec.scalar_tensor_tensor(osb[:, n * 512:(n + 1) * 512],
                                             res[m][:, n * 512:(n + 1) * 512],
                                             1.0, pss[m * 2 + n][:],
                                             AluOp.mult, AluOp.add)
                sync.dma_start(out_ext[m * 128:(m + 1) * 128, :], osb[:])
            s2ctx.close()

    if split_waits:
        split_excess_waits(nc)
    return nc


def prep_inputs(inputs):
    """Host-side sharding/weight prep.  Returns (a_vals, in_maps)."""
    f32 = lambda a: np.ascontiguousarray(np.asarray(a, np.float32))
    bf = lambda a: np.ascontiguousarray(np.asarray(a, np.float32).astype(BF16))

    A_f = -np.exp(f32(inputs["Alog_f"]))
    A_r = -np.exp(f32(inputs["Alog_r"]))
    assert np.abs(A_f - A_f[0:1]).max() < 1e-5, "A not d-independent"
    assert np.abs(A_f - A_r).max() < 1e-5, "A_f != A_r"
    a_vals = [float(v) for v in A_f[0]]

    x = f32(inputs["x"])
    w1 = f32(inputs["conv1_w"]); w2 = f32(inputs["conv2_w"]); w3 = f32(inputs["conv3_w"])
    w1T = bf(np.transpose(w1, (2, 1, 0)).reshape(3, 6, 128, 128))
    w2T = bf(np.transpose(w2, (2, 1, 0)).reshape(3, 1, 128, 256))
    w3T = bf(np.transpose(w3, (2, 1, 0)).reshape(3, 2, 128, 512))
    onehot = np.zeros((3, 128, 32), np.float32)
    for i, cg in enumerate((4, 8, 16)):
        onehot[i, np.arange(128), np.arange(128) // cg] = 1.0
    ln_g = f32(inputs["ln_g"]); ln_b = f32(inputs["ln_b"])
    ipw = f32(inputs["in_proj_w"])
    opw = f32(inputs["out_proj_w"])

    # full out_proj, ordered to match the AllGather layout: global row
    # j*128+c of ygall = d_inner index j*128+c (core-major x dtile ordering
    # equals natural order since core g owns rows [256g, 256g+256)).
    outT_full = bf(np.stack([opw[:, j * 128:(j + 1) * 128].T for j in range(8)]))

    common = dict(
        w1T=w1T, w2T=w2T, w3T=w3T,
        cb1=f32(inputs["conv1_b"]).reshape(128, 1),
        cb2=f32(inputs["conv2_b"]).reshape(256, 1),
        cb3=f32(inputs["conv3_b"]).reshape(512, 1),
        gng1=f32(inputs["gn1_g"]).reshape(128, 1),
        gnb1=f32(inputs["gn1_b"]).reshape(128, 1),
        gng2=f32(inputs["gn2_g"]).reshape(256, 1),
        gnb2=f32(inputs["gn2_b"]).reshape(256, 1),
        gng3=f32(inputs["gn3_g"]).reshape(512, 1),
        gnb3=f32(inputs["gn3_b"]).reshape(512, 1),
        onehot=bf(onehot),
        onehotT=np.ascontiguousarray(np.transpose(onehot, (0, 2, 1))),
        ones_col=bf(np.ones((128, 1), np.float32)),
        ident=bf(np.eye(128, dtype=np.float32)),
        outT=outT_full,
    )

    in_maps = []
    for core in range(NCORES):
        b, grp = core // NGRP, core % NGRP
        rows = np.arange(grp * DSH, (grp + 1) * DSH)
        sel = np.concatenate([rows, DI + rows])
        Wsel = ipw[sel] * ln_g[None, :]
        inprojT = bf(Wsel.T.reshape(4, 128, 2 * DSH))
        augTm = bf(np.stack([Wsel.sum(1), ipw[sel] @ ln_b]))
        xpTm = np.stack([
            bf(f32(inputs[f"xp_w_{s}"])[:, rows].T.reshape(2, 128, 64))
            for s in ("f", "r")])
        dtTm = np.stack([
            bf(f32(inputs[f"dt_w_{s}"])[rows].T) for s in ("f", "r")])
        ndtbm = np.stack([
            -f32(inputs[f"dt_b_{s}"])[rows].reshape(DSH, 1) for s in ("f", "r")])
        # diag conv-weight taps: cvdg[dir][dt][k] = diag(cv_w[rows dt-slice, k])
        cvdg = np.zeros((2, 2, 4, 128, 128), np.float32)
        cvbdg = np.zeros((2, 2, 128, 128), np.float32)
        Ddg = np.zeros((2, 2, 128, 128), np.float32)
        for di, sfx in enumerate(("f", "r")):
            wv = f32(inputs[f"cv_w_{sfx}"])[rows, 0]          # (256, 4)
            bv = f32(inputs[f"cv_b_{sfx}"])[rows]             # (256,)
            Dv = f32(inputs[f"D_{sfx}"])[rows]                # (256,)
            for dt in range(2):
                seg = slice(dt * 128, (dt + 1) * 128)
                for k in range(4):
                    np.fill_diagonal(cvdg[di, dt, k], wv[seg, k])
                np.fill_diagonal(cvbdg[di, dt], bv[seg])
                np.fill_diagonal(Ddg[di, dt], Dv[seg])
        xpadded = bf(np.pad(x[b], ((0, 0), (1, 1))))
        m = dict(common)
        m.update(x=xpadded, inprojT=inprojT, augT=augTm, xpT=xpTm, dtT=dtTm,
                 ndtb=ndtbm, cvdg=bf(cvdg), cvbdg=bf(cvbdg), Ddg=bf(Ddg))
        in_maps.append(m)
    return a_vals, in_maps


def kernel(**inputs) -> np.ndarray:
    from concourse.bass_utils import run_bass_kernel_spmd
    a_vals, in_maps = prep_inputs(inputs)
    nc = build_program(a_vals)
    res = run_bass_kernel_spmd(nc, in_maps, list(range(NCORES)))
    out = np.stack([res.results[0]["out"], res.results[NGRP]["out"]])
    return np.ascontiguousarray(out.astype(np.float32))


if __name__ == "__main__":
    import reference as R
    import jax
    with jax.default_device(jax.devices("cpu")[0]):
        inp = {k: np.asarray(v) for k, v in R.setup_inputs().items()}
        ref = np.asarray(R.reference(**R.setup_inputs()))
    got = kernel(**inp)
    err = np.abs(got - ref).max() / np.abs(ref).max()
    print("Relative error:", err)


# revision 33
# speedup vs baseline: 10.0370x; 10.0370x over previous
"""Trainium2 Bass kernel for nn_CLIP_Embedding_35613868818658.

CNN stem (3x conv1d+GroupNorm+ReLU, 768->128->256->512) -> LayerNorm ->
bidirectional Mamba (selective scan, d_inner=1024, d_state=16, L=1024) ->
out_proj + residual.  Output (2, 512, 1024) f32.

Sharding: 2 batch-groups x 4-way d_inner split (DSH=256 rows per core).
Cores 0-3 handle b=0, cores 4-7 handle b=1; core g within a group owns
d_inner rows [256g, 256(g+1)).  Cross-core traffic per group: one bf16
AllReduce of the x_dbl partials ([128,1024] = 256KB) and one bf16
AllGather of the gated scan outputs ([256,1024] per core); every core
then runs the full out_proj matmul locally (PE is idle) and writes the
final f32 output directly.

The selective scan runs as 16 (one per state index s) hardware
tensor_tensor_scan instructions per d-tile over a [128, 2048] layout that
concatenates the forward and (time-reversed) backward directions along the
free axis; a[, t=0|1024] = 0 resets the recurrence at segment starts.
The per-state y accumulation (y += C_s*h_s) and the D skip-term run on the
tensor engine as identity/diagonal matmul accumulation into PSUM; the
depthwise conv runs as 4 diagonal-matmul taps + Silu-from-PSUM.
"""

import numpy as np
import ml_dtypes

import concourse.bass as bass
import concourse.mybir as mybir
import concourse.tile as tile
from contextlib import ExitStack

BF16 = ml_dtypes.bfloat16
F32 = mybir.dt.float32
BF = mybir.dt.bfloat16

B, CIN, L = 2, 768, 1024
DM, DI, DS, DTR, DC = 512, 1024, 16, 32, 4
NCORES, NGRP = 8, 4
DSH = DI // NGRP          # 256 d_inner rows per core
NDT = DSH // 128          # 2 d-tiles of 128 partitions
T2 = 2 * L                # fwd|rev concatenated time axis
EPS = 1e-5

AluOp = mybir.AluOpType
ActFn = mybir.ActivationFunctionType


def _ap_bcast_dram(handle, offset, dims):
    """Raw AP on a DRAM tensor: dims is a list of [step, count]."""
    return bass.AP(tensor=handle, offset=offset, ap=[list(d) for d in dims])


def split_excess_waits(nc, max_waits=1):
    """Walrus rejects instructions carrying more sync waits than the ISA
    encoding has slots for (1 on this toolchain).  Move excess waits onto
    preceding same-engine NoOps."""
    for bb in nc.main_func.blocks:
        insts = bb.instructions
        out, changed = [], False
        for ins in insts:
            si = ins.sync_info
            if si is not None and si.on_wait is not None and len(si.on_wait) > max_waits:
                waits = list(si.on_wait)
                keep, rest = waits[:max_waits], waits[max_waits:]
                idx = 0
                while rest:
                    chunk, rest = rest[:max_waits], rest[max_waits:]
                    nop = mybir.InstNoOp(
                        name=f"{ins.name}-wsplit{idx}",
                        engine=ins.engine,
                        sync_info=mybir.SyncInfo(on_wait=chunk, on_update=[]),
                        bass_nofuse=True,
                    )
                    out.append(nop)
                    idx += 1
                ins.sync_info = mybir.SyncInfo(
                    on_wait=keep, on_update=list(si.on_update or [])
                )
                changed = True
            out.append(ins)
        if changed:
            bb.instructions = out


def build_program(a_vals, split_waits=True, reps=1):
    """Build the SPMD Bass program.  a_vals: 16 negative floats, A[s] = -(s+1)
    (verified d-independent and equal for both directions on the host)."""
    nc = bass.Bass("TRN2", target_bir_lowering=False, debug=False,
                   num_devices=NCORES)

    dt_in = lambda n, s, d=BF: nc.dram_tensor(n, list(s), d, kind="ExternalInput")

    x_in = dt_in("x", (CIN, L + 2))                      # host-padded, bf16
    w1T = dt_in("w1T", (3, 6, 128, 128))
    w2T = dt_in("w2T", (3, 1, 128, 256))
    w3T = dt_in("w3T", (3, 2, 128, 512))
    cb1 = dt_in("cb1", (128, 1), F32)
    cb2 = dt_in("cb2", (256, 1), F32)
    cb3 = dt_in("cb3", (512, 1), F32)
    gng1 = dt_in("gng1", (128, 1), F32)
    gnb1 = dt_in("gnb1", (128, 1), F32)
    gng2 = dt_in("gng2", (256, 1), F32)
    gnb2 = dt_in("gnb2", (256, 1), F32)
    gng3 = dt_in("gng3", (512, 1), F32)
    gnb3 = dt_in("gnb3", (512, 1), F32)
    onehot = dt_in("onehot", (3, 128, 32))
    onehotT = dt_in("onehotT", (3, 32, 128), F32)
    ones_col = dt_in("ones_col", (128, 1))
    inprojT = dt_in("inprojT", (4, 128, 512))
    augT = dt_in("augT", (2, 512))
    xpT = dt_in("xpT", (2, 2, 128, 64))                 # [dir][ktile]
    dtT = dt_in("dtT", (2, 32, 256))                    # [dir]
    ndtb = dt_in("ndtb", (2, 256, 1), F32)              # -dt_b
    cvdg = dt_in("cvdg", (2, 2, 4, 128, 128))           # [dir][dt][tap] diag
    cvbdg = dt_in("cvbdg", (2, 2, 128, 128))            # [dir][dt] diag(cv_b)
    Ddg = dt_in("Ddg", (2, 2, 128, 128))                # [dir][dt] diag(D)
    ident = dt_in("ident", (128, 128))
    outT = dt_in("outT", (8, 128, 512))                 # full d_inner

    out_ext = nc.dram_tensor("out", [DM, L], F32, kind="ExternalOutput")

    with tile.TileContext(nc) as tc, ExitStack() as ctx:
        P = 128
        consts = ctx.enter_context(tc.tile_pool(name="consts", bufs=1))
        mid = ctx.enter_context(tc.tile_pool(name="mid", bufs=1))
        dram = ctx.enter_context(tc.tile_pool(name="dram", bufs=1, space="DRAM"))
        sync, vec, pool, act, pe = nc.sync, nc.vector, nc.gpsimd, nc.scalar, nc.tensor

        # ---------------- consts to SBUF ----------------
        def load(poolh, shape, src, dtype=BF, name=None):
            t = poolh.tile(list(shape), dtype, tag=name)
            sync.dma_start(t[:], src)
            return t

        w1 = [[load(consts, (P, 128), w1T[k, ct], name=f"w1_{k}_{ct}")
               for ct in range(6)] for k in range(3)]
        w2 = [[load(consts, (P, 256), w2T[k, ct], name=f"w2_{k}_{ct}")
               for ct in range(1)] for k in range(3)]
        w3 = [[load(consts, (P, 512), w3T[k, ct], name=f"w3_{k}_{ct}")
               for ct in range(2)] for k in range(3)]
        def load_cols(dramt, co, name, width=1):
            return [load(consts, (128, width), dramt[mt * 128:(mt + 1) * 128, :],
                         F32, f"{name}{mt}") for mt in range(co // 128)]

        cbs = [load_cols(cb1, 128, "cb1"), load_cols(cb2, 256, "cb2"),
               load_cols(cb3, 512, "cb3")]
        gngs = [load_cols(gng1, 128, "gng1"), load_cols(gng2, 256, "gng2"),
                load_cols(gng3, 512, "gng3")]
        gnbs = [load_cols(gnb1, 128, "gnb1"), load_cols(gnb2, 256, "gnb2"),
                load_cols(gnb3, 512, "gnb3")]
        oneh = [load(consts, (P, 32), onehot[i], name=f"onehot{i}")
                for i in range(3)]
        ohT = [load(consts, (32, 128), onehotT[i], F32, name=f"onehotT{i}")
               for i in range(3)]
        ones1 = load(consts, (P, 1), ones_col[:], name="ones1")
        ones_1xP = consts.tile([1, P], BF, tag="ones_1xP")
        vec.memset(ones_1xP[:], 1.0)
        ipT = [load(consts, (P, 512), inprojT[kt], name=f"ipT{kt}") for kt in range(4)]
        augTs = load(consts, (2, 512), augT[:], name="augT")
        xpTs = [[load(consts, (P, 64), xpT[d, kt], name=f"xpT{d}{kt}")
                 for kt in range(2)] for d in range(2)]
        dtTs = [load(consts, (32, 256), dtT[d], name=f"dtT{d}") for d in range(2)]
        ndtbs = [[load(consts, (128, 1), ndtb[d, dt * 128:(dt + 1) * 128, :], F32,
                       f"ndtb{d}{dt}") for dt in range(2)] for d in range(2)]
        cvds = [[[load(consts, (P, 128), cvdg[d, dt, k], name=f"cvd{d}{dt}{k}")
                  for k in range(4)] for dt in range(2)] for d in range(2)]
        cvbds = [[load(consts, (P, 128), cvbdg[d, dt], name=f"cvbd{d}{dt}")
                  for dt in range(2)] for d in range(2)]
        ones_row = consts.tile([P, 512], BF, tag="ones_row")
        vec.memset(ones_row[:], 1.0)
        Ddgs = [[load(consts, (P, 128), Ddg[d, dt], name=f"Ddg{d}{dt}")
                 for dt in range(2)] for d in range(2)]
        idn = load(consts, (P, 128), ident[:], name="ident")
        outTs = [load(consts, (P, 512), outT[j], name=f"outT{j}") for j in range(8)]

        epsc = consts.tile([128, 1], F32, tag="epsc")
        vec.memset(epsc[:], EPS)

        # DRAM scratch
        xdbl_loc = dram.tile([128, L], BF, tag="xdbl_loc")
        xdbl_gat = dram.tile([NGRP * 128, L], BF, tag="xdbl_gat")
        xdbl_red = dram.tile([128, L], BF, tag="xdbl_red")
        ygl = dram.tile([2 * P, L], BF, tag="ygl")
        ygall = dram.tile([NGRP * 2 * P, L], BF, tag="ygall")

        for rep in range(reps):
            fctx = ExitStack()
            psum = fctx.enter_context(tc.tile_pool(name=f"psum{rep}", bufs=2,
                                                   space="PSUM"))
            stem = fctx.enter_context(tc.tile_pool(name=f"stem{rep}", bufs=1))
            stemtmp = fctx.enter_context(tc.tile_pool(name=f"stemtmp{rep}", bufs=3))
            statp = fctx.enter_context(tc.tile_pool(name=f"statp{rep}", bufs=2))
            rows = fctx.enter_context(tc.tile_pool(name=f"rows{rep}", bufs=1))
            x_t = [load(stem, (P, L + 2), x_in[ct * P:(ct + 1) * P, :],
                        name=f"x{ct}") for ct in range(6)]
            # ---------------- CNN stem ----------------
            def conv_gn_relu(layer, in_tiles, ws, cb, gng, gnb, co, out_f32):
                """in_tiles: list of padded (128, L+2) bf16; returns list of
                normalized+relu'd output tiles.  out_f32: emit f32 (for res)."""
                n_ct = len(in_tiles)
                n_co = co // 128
                cg = co // 32            # channels per group
                ngt = 128 // cg          # groups per 128-channel tile
                group_elems = float(cg) * L
                outs = []
                for mt in range(n_co):
                    h_raw = stemtmp.tile([P, L], F32, tag="h_raw")
                    stat4 = statp.tile([P, 4], F32, tag="stat4")
                    sq = stemtmp.tile([P, 512], BF, tag="sq")
                    for n in range(2):
                        ps = psum.tile([P, 512], F32, tag="ps_main", name="ps")
                        nmm = n_ct * 3
                        i = 0
                        for ct in range(n_ct):
                            for k in range(3):
                                pe.matmul(
                                    ps[:],
                                    ws[k][ct][:, mt * 128:(mt + 1) * 128],
                                    in_tiles[ct][:, n * 512 + k: n * 512 + k + 512],
                                    start=(i == 0), stop=(i == nmm - 1),
                                )
                                i += 1
                        act.activation(h_raw[:, n * 512:(n + 1) * 512], ps[:],
                                       ActFn.Identity, bias=cb[mt][:],
                                       accum_out=stat4[:, n:n + 1])
                        act.activation(sq[:], h_raw[:, n * 512:(n + 1) * 512],
                                       ActFn.Square, accum_out=stat4[:, 2 + n:3 + n])
                    # group stats: per-partition sums -> per-group via one-hot matmul
                    stat4b = statp.tile([P, 4], BF, tag="stat4b")
                    vec.tensor_copy(stat4b[:], stat4[:])
                    gps = psum.tile([32, 4], F32, tag="ps_small", name="gps", bufs=2)
                    pe.matmul(gps[:], oneh[layer - 1][:], stat4b[:])
                    gsb = statp.tile([32, 4], F32, tag="gsb")
                    vec.tensor_copy(gsb[:], gps[:])
                    sx = statp.tile([32, 1], F32, tag="sx")
                    sq_g = statp.tile([32, 1], F32, tag="sq_g")
                    vec.tensor_add(sx[:], gsb[:, 0:1], gsb[:, 1:2])
                    vec.tensor_add(sq_g[:], gsb[:, 2:3], gsb[:, 3:4])
                    mean = statp.tile([32, 1], F32, tag="mean")
                    vec.tensor_scalar_mul(mean[:], sx[:], 1.0 / group_elems)
                    msq = statp.tile([32, 1], F32, tag="msq")
                    vec.tensor_mul(msq[:], mean[:], mean[:])
                    var = statp.tile([32, 1], F32, tag="var")
                    vec.scalar_tensor_tensor(var[:], sq_g[:], 1.0 / group_elems, msq[:],
                                             AluOp.mult, AluOp.subtract)
                    sig_g = statp.tile([32, 1], F32, tag="sig_g")
                    act.activation(sig_g[:], var[:], ActFn.Sqrt, bias=epsc[:32, :])
                    rstd = statp.tile([32, 1], F32, tag="rstd")
                    vec.reciprocal(rstd[:], sig_g[:])
                    # pack [rstd, mean] and expand groups 32 -> channels 128
                    # via a one-hot-transpose matmul (no DRAM round trip)
                    stat2 = statp.tile([32, 2], F32, tag="stat2")
                    vec.tensor_copy(stat2[:, 0:1], rstd[:])
                    vec.tensor_copy(stat2[:, 1:2], mean[:])
                    gch = psum.tile([P, 2], F32, tag="ps_bc", name="gch", bufs=2)
                    pe.matmul(gch[:], ohT[layer - 1][:], stat2[:])
                    ch2 = statp.tile([P, 2], F32, tag="ch2")
                    act.activation(ch2[:], gch[:], ActFn.Copy)
                    scale_c = statp.tile([P, 1], F32, tag="scale_c")
                    vec.tensor_mul(scale_c[:], ch2[:, 0:1], gng[mt][:])
                    nmean_s = statp.tile([P, 1], F32, tag="nmean_s")
                    vec.tensor_mul(nmean_s[:], ch2[:, 1:2], scale_c[:])
                    bias_c = statp.tile([P, 1], F32, tag="bias_c")
                    vec.tensor_sub(bias_c[:], gnb[mt][:], nmean_s[:])
                    if out_f32:
                        h_out = mid.tile([P, L], F32, tag=f"res{mt}")
                        act.activation(h_out[:], h_raw[:], ActFn.Relu,
                                       scale=scale_c[:], bias=bias_c[:])
                    else:
                        h_out = stem.tile([P, L + 2], BF, tag=f"h{layer}_{mt}")
                        vec.memset(h_out[:, 0:1], 0.0)
                        vec.memset(h_out[:, L + 1:L + 2], 0.0)
                        act.activation(h_out[:, 1:L + 1], h_raw[:], ActFn.Relu,
                                       scale=scale_c[:], bias=bias_c[:])
                    outs.append(h_out)
                return outs

            h1 = conv_gn_relu(1, x_t, w1, cbs[0], gngs[0], gnbs[0], 128, False)
            h2 = conv_gn_relu(2, h1, w2, cbs[1], gngs[1], gnbs[1], 256, False)
            res = conv_gn_relu(3, h2, w3, cbs[2], gngs[2], gnbs[2], 512, True)

            h3b = []
            for mt in range(4):
                t = stem.tile([P, L], BF, tag=f"h3b{mt}")
                vec.tensor_copy(t[:], res[mt][:])
                h3b.append(t)

            # ---------------- LayerNorm stats (over channels, via matmuls) -------
            hsq = []
            for mt in range(4):
                t = stemtmp.tile([P, L], BF, tag="hsq")
                vec.tensor_mul(t[:], h3b[mt][:], h3b[mt][:])
                hsq.append(t)
            musum = rows.tile([1, L], F32, tag="musum")
            sqsum = rows.tile([1, L], F32, tag="sqsum")
            for n in range(2):
                mu_ps = psum.tile([1, 512], F32, tag="ps_row", name="mu_ps", bufs=2)
                for kt in range(4):
                    pe.matmul(mu_ps[:], ones1[:],
                              h3b[kt][:, n * 512:(n + 1) * 512],
                              start=(kt == 0), stop=(kt == 3))
                act.activation(musum[:, n * 512:(n + 1) * 512], mu_ps[:], ActFn.Copy)
                sq_ps = psum.tile([1, 512], F32, tag="ps_row", name="sq_ps", bufs=2)
                for kt in range(4):
                    pe.matmul(sq_ps[:], ones1[:],
                              hsq[kt][:, n * 512:(n + 1) * 512],
                              start=(kt == 0), stop=(kt == 3))
                act.activation(sqsum[:, n * 512:(n + 1) * 512], sq_ps[:], ActFn.Copy)
            nmu = rows.tile([1, L], F32, tag="nmu")
            vec.tensor_scalar_mul(nmu[:], musum[:], -1.0 / DM)
            msql = rows.tile([1, L], F32, tag="msql")
            act.activation(msql[:], musum[:], ActFn.Square, scale=1.0 / DM)
            varl = rows.tile([1, L], F32, tag="varl")
            vec.scalar_tensor_tensor(varl[:], sqsum[:], 1.0 / DM, msql[:],
                                     AluOp.mult, AluOp.subtract)
            sigma = rows.tile([1, L], F32, tag="sigma")
            act.activation(sigma[:], varl[:], ActFn.Sqrt, bias=epsc[:1, :])
            recip = rows.tile([1, L], F32, tag="recip")
            vec.reciprocal(recip[:], sigma[:])
            nmu_b = rows.tile([1, L], BF, tag="nmu_b")
            vec.tensor_copy(nmu_b[:], nmu[:])
            sig_b = rows.tile([1, L], BF, tag="sig_b")
            vec.tensor_copy(sig_b[:], sigma[:])
            aug = rows.tile([2, L], BF, tag="aug")
            sync.dma_start(aug[0:1, :], nmu_b[:])
            sync.dma_start(aug[1:2, :], sig_b[:])
            recip_b = rows.tile([1, L], BF, tag="recip_b")
            vec.tensor_copy(recip_b[:], recip[:])
            rbc = rows.tile([P, L], BF, tag="rbc")
            for n in range(2):
                rps = psum.tile([P, 512], F32, tag="ps_main", name="rps")
                pe.matmul(rps[:], ones_1xP[:], recip_b[:, n * 512:(n + 1) * 512])
                act.activation(rbc[:, n * 512:(n + 1) * 512], rps[:], ActFn.Copy)

            # ---------------- in_proj (LN folded in) ----------------
            # xpad[dt]: (128, L+6) bf16, 3 zero cols each side; z[dt]: (128, L)
            xpad = []
            zt = []
            for dt in range(NDT):
                xp_ = mid.tile([P, L + 6], BF, tag=f"xpad{dt}")
                vec.memset(xp_[:, 0:3], 0.0)
                vec.memset(xp_[:, L + 3:L + 6], 0.0)
                xpad.append(xp_)
                zt.append(mid.tile([P, L], BF, tag=f"z{dt}", name=f"z{dt}"))
            for m in range(4):
                for n in range(2):
                    ps = psum.tile([P, 512], F32, tag="ps_main", name="ps")
                    for kt in range(4):
                        pe.matmul(ps[:], ipT[kt][:, m * 128:(m + 1) * 128],
                                  h3b[kt][:, n * 512:(n + 1) * 512],
                                  start=(kt == 0), stop=False)
                    pe.matmul(ps[:], augTs[:, m * 128:(m + 1) * 128],
                              aug[:, n * 512:(n + 1) * 512], start=False, stop=True)
                    if m < 2:
                        dst = xpad[m][:, 3 + n * 512: 3 + (n + 1) * 512]
                    else:
                        dst = zt[m - 2][:, n * 512:(n + 1) * 512]
                    vec.tensor_mul(dst, ps[:], rbc[:, n * 512:(n + 1) * 512])

            fctx.close()  # free stem/LN scratch (incl. psum) for the scan phase
            s1ctx = ExitStack()
            psum1 = s1ctx.enter_context(tc.tile_pool(name=f"psum1_{rep}", bufs=1,
                                                     space="PSUM"))
            scanp = s1ctx.enter_context(tc.tile_pool(name=f"scanp{rep}", bufs=2))
            onep = s1ctx.enter_context(tc.tile_pool(name=f"onep{rep}", bufs=1))

            # ------- depthwise causal conv (PE diag taps) + silu-from-PSUM -------
            u_cat = [mid.tile([P, T2], BF, tag=f"u{dt}", name=f"u{dt}")
                     for dt in range(NDT)]
            for dt in range(NDT):
                for d in range(2):  # 0 = fwd, 1 = rev (tau domain)
                    pdw = psum1.tile([P, L], F32, tag="ps_dw", name="pdw", bufs=2)
                    sg = scanp.tile([P, L], BF, tag="dwsg")
                    for c in range(2):
                        pe.matmul(pdw[:, c * 512:(c + 1) * 512],
                                  cvbds[d][dt][:], ones_row[:],
                                  start=True, stop=False)
                        for k in range(4):
                            off = (k if d == 0 else 3 - k) + c * 512
                            pe.matmul(pdw[:, c * 512:(c + 1) * 512],
                                      cvds[d][dt][k][:],
                                      xpad[dt][:, off:off + 512],
                                      start=False, stop=(k == 3))
                        act.activation(sg[:, c * 512:(c + 1) * 512],
                                       pdw[:, c * 512:(c + 1) * 512],
                                       ActFn.Sigmoid)
                    if d == 0:
                        vec.tensor_mul(u_cat[dt][:, 0:L], pdw[:], sg[:])
                    else:
                        tmpv = scanp.tile([P, L], BF, tag="dwtmp")
                        vec.tensor_mul(tmpv[:], pdw[:], sg[:])
                        vec.tensor_copy(u_cat[dt][:, L:T2], tmpv[:, L - 1::-1])

            # ---------------- x_dbl projection + bf16 AllReduce ----------------
            xsb = onep.tile([128, L], BF, tag="xsb")
            for d in range(2):
                for n in range(2):
                    xps = psum1.tile([64, 512], F32, tag="ps_xp", name="xps")
                    for dt in range(NDT):
                        pe.matmul(xps[:], xpTs[d][dt][:],
                                  u_cat[dt][:, d * L + n * 512: d * L + (n + 1) * 512],
                                  start=(dt == 0), stop=(dt == 1))
                    act.activation(xsb[64 * d:64 * d + 64, n * 512:(n + 1) * 512],
                                   xps[:], ActFn.Copy)
            sync.dma_start(xdbl_loc[:], xsb[:])
            pool.collective_compute(
                "AllGather", AluOp.bypass,
                replica_groups=[[0, 1, 2, 3], [4, 5, 6, 7]],
                ins=[xdbl_loc[:].opt()],
                outs=[xdbl_gat[:].opt()],
            )
            # sum the 4 gathered partials locally (cheaper than AllReduce's
            # ring latency floor), then park the result back in DRAM for the
            # per-state partition-broadcast reads.
            xgp = []
            for g in range(NGRP):
                t = onep.tile([128, L], BF, tag=f"xgp{g}", name=f"xgp{g}")
                (sync if g % 2 == 0 else act).dma_start(
                    t[:], xdbl_gat[g * 128:(g + 1) * 128, :])
                xgp.append(t)
            xs01 = onep.tile([128, L], BF, tag="xs01")
            vec.tensor_add(xs01[:], xgp[0][:], xgp[1][:])
            xs23 = onep.tile([128, L], BF, tag="xs23")
            pool.tensor_add(xs23[:], xgp[2][:], xgp[3][:])
            xsum = onep.tile([128, L], BF, tag="xsum")
            vec.tensor_add(xsum[:], xs01[:], xs23[:])
            sync.dma_start(xdbl_red[:], xsum[:])

            # ------- dt_proj -> m = -softplus(dt @ dtw + dt_b) = ln(sigmoid(-x))
            m_cat = [mid.tile([P, T2], BF, tag=f"m{dt}", name=f"m{dt}")
                     for dt in range(NDT)]
            dtf1 = onep.tile([32, L], BF, tag="dtf1")
            sync.dma_start(dtf1[:], xsum[64:96, :])
            for dt in range(NDT):
                for d in range(2):
                    for n in range(2):
                        rhs = (xsum[0:32, n * 512:(n + 1) * 512] if d == 0
                               else dtf1[:, n * 512:(n + 1) * 512])
                        ps = psum1.tile([P, 512], F32, tag="ps_dt", name="psdt")
                        pe.matmul(ps[:], dtTs[d][:, dt * 128:(dt + 1) * 128],
                                  rhs)
                        sgm = scanp.tile([P, 512], F32, tag="sgm")
                        act.activation(sgm[:], ps[:], ActFn.Sigmoid, scale=-1.0,
                                       bias=ndtbs[d][dt][:])
                        act.activation(m_cat[dt][:, d * L + n * 512: d * L + (n + 1) * 512],
                                       sgm[:], ActFn.Ln)

            # mx = -(m * u) = delta * u
            mx = [mid.tile([P, T2], BF, tag=f"mx{dt}", name=f"mx{dt}")
                  for dt in range(NDT)]
            for dt in range(NDT):
                vec.scalar_tensor_tensor(mx[dt][:], m_cat[dt][:], -1.0, u_cat[dt][:],
                                         AluOp.mult, AluOp.mult)

            # z gating (independent of the scan)
            zs = []
            for dt in range(NDT):
                sgz = scanp.tile([P, L], BF, tag="sgz")
                act.activation(sgz[:], zt[dt][:], ActFn.Sigmoid)
                t = mid.tile([P, L], BF, tag=f"zs{dt}")
                vec.tensor_mul(t[:], zt[dt][:], sgz[:])
                zs.append(t)

            s1ctx.close()
            s2ctx = ExitStack()
            scan2 = s2ctx.enter_context(tc.tile_pool(name=f"scan2_{rep}", bufs=2))
            psy_ctx = ExitStack()
            psum2 = psy_ctx.enter_context(tc.tile_pool(name=f"psum2_{rep}", bufs=1,
                                                       space="PSUM"))

            # ---------------- selective scan ----------------
            xr_ap = xdbl_red[:]
            ps_y = [psum2.tile([P, T2], F32, tag=f"ps_y{dt}", name=f"ps_y{dt}",
                               bufs=1) for dt in range(NDT)]
            for s in range(16):
                Bs = scan2.tile([P, T2], BF, tag="Bs")
                sync.dma_start(
                    Bs[:],
                    _ap_bcast_dram(xr_ap.tensor, xr_ap.offset + (32 + s) * L,
                                   [[0, P], [64 * L, 2], [1, L]]),
                )
                Cs = scan2.tile([P, T2], BF, tag="Cs")
                sync.dma_start(
                    Cs[:],
                    _ap_bcast_dram(xr_ap.tensor, xr_ap.offset + (48 + s) * L,
                                   [[0, P], [64 * L, 2], [1, L]]),
                )
                for dt in range(NDT):
                    a_s = scan2.tile([P, T2], BF, tag="a_s")
                    act.activation(a_s[:], m_cat[dt][:], ActFn.Exp,
                                   scale=float(-a_vals[s]))
                    vec.memset(a_s[:, 0:1], 0.0)
                    vec.memset(a_s[:, L:L + 1], 0.0)
                    b_s = scan2.tile([P, T2], BF, tag="b_s")
                    bs_eng = pool if dt == 0 else vec
                    bs_eng.tensor_mul(b_s[:], mx[dt][:], Bs[:])
                    h_s = scan2.tile([P, T2], BF, tag="h_s")
                    vec.tensor_tensor_scan(h_s[:], a_s[:], b_s[:], 0.0,
                                           AluOp.mult, AluOp.add)
                    gs = scan2.tile([P, T2], BF, tag="gs")
                    pool.tensor_mul(gs[:], h_s[:], Cs[:])
                    for c in range(4):
                        pe.matmul(ps_y[dt][:, c * 512:(c + 1) * 512], idn[:],
                                  gs[:, c * 512:(c + 1) * 512],
                                  start=(s == 0), stop=False)
            # D skip-term: ps_y += diag(D_dir) @ u (fwd half / rev half)
            for dt in range(NDT):
                for c in range(4):
                    d = c // 2
                    pe.matmul(ps_y[dt][:, c * 512:(c + 1) * 512], Ddgs[d][dt][:],
                              u_cat[dt][:, c * 512:(c + 1) * 512],
                              start=False, stop=True)

            # ---------------- combine directions, gate, AllGather ----------------
            for dt in range(NDT):
                yf = scan2.tile([P, L], BF, tag="yf")
                vec.tensor_copy(yf[:], ps_y[dt][:, 0:L])
                ysum = scan2.tile([P, L], BF, tag="ysum")
                vec.tensor_add(ysum[:], yf[:], ps_y[dt][:, T2 - 1:L - 1:-1])
                yg = scan2.tile([P, L], BF, tag="yg")
                vec.tensor_mul(yg[:], ysum[:], zs[dt][:])
                sync.dma_start(ygl[dt * P:(dt + 1) * P, :], yg[:])
            psy_ctx.close()
            psum3 = s2ctx.enter_context(tc.tile_pool(name=f"psum3_{rep}", bufs=1,
                                                     space="PSUM"))
            pool.collective_compute(
                "AllGather", AluOp.bypass,
                replica_groups=[[0, 1, 2, 3], [4, 5, 6, 7]],
                ins=[ygl[:].opt()],
                outs=[ygall[:].opt()],
            )

            # ---------------- full out_proj + residual on every core -----------
            # j-outer accumulation into 8 open PSUM groups so matmuls start as
            # soon as each gathered d-slice lands.
            ygf = [scan2.tile([P, L], BF, tag=f"ygf{j}", name=f"ygf{j}", bufs=1)
                   for j in range(8)]
            for j in range(8):
                eng = sync if j % 2 == 0 else act
                eng.dma_start(ygf[j][:], ygall[j * P:(j + 1) * P, :])
            pss = [psum3.tile([P, 512], F32, tag=f"po{q}", name=f"po{q}", bufs=1)
                   for q in range(8)]
            for j in range(8):
                for m in range(4):
                    for n in range(2):
                        pe.matmul(pss[m * 2 + n][:],
                                  outTs[j][:, m * 128:(m + 1) * 128],
                                  ygf[j][:, n * 512:(n + 1) * 512],
                                  start=(j == 0), stop=(j == 7))
            for m in range(4):
                osb = scan2.tile([P, L], F32, tag="osb")
                for n in range(2):
                    vec.scalar_tensor_tensor(osb[:, n * 512:(n + 1) * 512],
                                             res[m][:, n * 512:(n + 1) * 512],
                                             1.0, pss[m * 2 + n][:],
                                             AluOp.mult, AluOp.add)
                sync.dma_start(out_ext[m * 128:(m + 1) * 128, :], osb[:])
            s2ctx.close()

    if split_waits:
        split_excess_waits(nc)
    return nc


def prep_inputs(inputs):
    """Host-side sharding/weight prep.  Returns (a_vals, in_maps)."""
    f32 = lambda a: np.ascontiguousarray(np.asarray(a, np.float32))
    bf = lambda a: np.ascontiguousarray(np.asarray(a, np.float32).astype(BF16))

    A_f = -np.exp(f32(inputs["Alog_f"]))
    A_r = -np.exp(f32(inputs["Alog_r"]))
    assert np.abs(A_f - A_f[0:1]).max() < 1e-5, "A not d-independent"
    assert np.abs(A_f - A_r).max() < 1e-5, "A_f != A_r"
    a_vals = [float(v) for v in A_f[0]]

    x = f32(inputs["x"])
    w1 = f32(inputs["conv1_w"]); w2 = f32(inputs["conv2_w"]); w3 = f32(inputs["conv3_w"])
    w1T = bf(np.transpose(w1, (2, 1, 0)).reshape(3, 6, 128, 128))
    w2T = bf(np.transpose(w2, (2, 1, 0)).reshape(3, 1, 128, 256))
    w3T = bf(np.transpose(w3, (2, 1, 0)).reshape(3, 2, 128, 512))
    onehot = np.zeros((3, 128, 32), np.float32)
    for i, cg in enumerate((4, 8, 16)):
        onehot[i, np.arange(128), np.arange(128) // cg] = 1.0
    ln_g = f32(inputs["ln_g"]); ln_b = f32(inputs["ln_b"])
    ipw = f32(inputs["in_proj_w"])
    opw = f32(inputs["out_proj_w"])

    # full out_proj, ordered to match the AllGather layout: global row
    # j*128+c of ygall = d_inner index j*128+c (core-major x dtile ordering
    # equals natural order since core g owns rows [256g, 256g+256)).
    outT_full = bf(np.stack([opw[:, j * 128:(j + 1) * 128].T for j in range(8)]))

    common = dict(
        w1T=w1T, w2T=w2T, w3T=w3T,
        cb1=f32(inputs["conv1_b"]).reshape(128, 1),
        cb2=f32(inputs["conv2_b"]).reshape(256, 1),
        cb3=f32(inputs["conv3_b"]).reshape(512, 1),
        gng1=f32(inputs["gn1_g"]).reshape(128, 1),
        gnb1=f32(inputs["gn1_b"]).reshape(128, 1),
        gng2=f32(inputs["gn2_g"]).reshape(256, 1),
        gnb2=f32(inputs["gn2_b"]).reshape(256, 1),
        gng3=f32(inputs["gn3_g"]).reshape(512, 1),
        gnb3=f32(inputs["gn3_b"]).reshape(512, 1),
        onehot=bf(onehot),
        onehotT=np.ascontiguousarray(np.transpose(onehot, (0, 2, 1))),
        ones_col=bf(np.ones((128, 1), np.float32)),
        ident=bf(np.eye(128, dtype=np.float32)),
        outT=outT_full,
    )

    in_maps = []
    for core in range(NCORES):
        b, grp = core // NGRP, core % NGRP
        rows = np.arange(grp * DSH, (grp + 1) * DSH)
        sel = np.concatenate([rows, DI + rows])
        Wsel = ipw[sel] * ln_g[None, :]
        inprojT = bf(Wsel.T.reshape(4, 128, 2 * DSH))
        augTm = bf(np.stack([Wsel.sum(1), ipw[sel] @ ln_b]))
        xpTm = np.stack([
            bf(f32(inputs[f"xp_w_{s}"])[:, rows].T.reshape(2, 128, 64))
            for s in ("f", "r")])
        dtTm = np.stack([
            bf(f32(inputs[f"dt_w_{s}"])[rows].T) for s in ("f", "r")])
        ndtbm = np.stack([
            -f32(inputs[f"dt_b_{s}"])[rows].reshape(DSH, 1) for s in ("f", "r")])
        # diag conv-weight taps: cvdg[dir][dt][k] = diag(cv_w[rows dt-slice, k])
        cvdg = np.zeros((2, 2, 4, 128, 128), np.float32)
        cvbdg = np.zeros((2, 2, 128, 128), np.float32)
        Ddg = np.zeros((2, 2, 128, 128), np.float32)
        for di, sfx in enumerate(("f", "r")):
            wv = f32(inputs[f"cv_w_{sfx}"])[rows, 0]          # (256, 4)
            bv = f32(inputs[f"cv_b_{sfx}"])[rows]             # (256,)
            Dv = f32(inputs[f"D_{sfx}"])[rows]                # (256,)
            for dt in range(2):
                seg = slice(dt * 128, (dt + 1) * 128)
                for k in range(4):
                    np.fill_diagonal(cvdg[di, dt, k], wv[seg, k])
                np.fill_diagonal(cvbdg[di, dt], bv[seg])
                np.fill_diagonal(Ddg[di, dt], Dv[seg])
        xpadded = bf(np.pad(x[b], ((0, 0), (1, 1))))
        m = dict(common)
        m.update(x=xpadded, inprojT=inprojT, augT=augTm, xpT=xpTm, dtT=dtTm,
                 ndtb=ndtbm, cvdg=bf(cvdg), cvbdg=bf(cvbdg), Ddg=bf(Ddg))
        in_maps.append(m)
    return a_vals, in_maps


def kernel(**inputs) -> np.ndarray:
    from concourse.bass_utils import run_bass_kernel_spmd
    a_vals, in_maps = prep_inputs(inputs)
    nc = build_program(a_vals)
    res = run_bass_kernel_spmd(nc, in_maps, list(range(NCORES)))
    out = np.stack([res.results[0]["out"], res.results[NGRP]["out"]])
    return np.ascontiguousarray(out.astype(np.float32))


if __name__ == "__main__":
    import reference as R
    import jax
    with jax.default_device(jax.devices("cpu")[0]):
        inp = {k: np.asarray(v) for k, v in R.setup_inputs().items()}
        ref = np.asarray(R.reference(**R.setup_inputs()))
    got = kernel(**inp)
    err = np.abs(got - ref).max() / np.abs(ref).max()
    print("Relative error:", err)
